# revision 27
# baseline (speedup 1.0000x reference)
"""Cross-attention kernel for Trainium2 (8 NeuronCores, SPMD).

Problem: q [2, 2048, 16, 64], kv [2, 2048, 2, 16, 64] (k=kv[:,:,0], v=kv[:,:,1])
  scores = einsum('bthd,bshd->bhts', q, k/sqrt(d)); P = softmax(scores, -1)
  out = einsum('bhts,bshd->bthd', P, v)    -> [2, 2048, 16, 64]

Sharding: 32 (b,h) heads across 8 cores -> 4 heads/core (data parallel on b,
tensor parallel on h; no communication).

Per-core algorithm (per head, t=s=2048, d=64), v3 pipeline:
  - Host lays out one fp16 tensor per head: Q^T [64,2048] duplicated into both
    PE row halves, K^T*scale packed (even s-tiles partitions 0-63, odd 64-127;
    2-way row-packed score matmuls), V' = [V, 1] per s-tile (ones column gives
    the softmax denominator for free in the same matmul).
  - 128 slots = (head, t-quarter, s-tile-pair). Per slot: two row-packed
    matmuls -> S^T halves of one [128,1024] PSUM tile; exp -> P^T fp16.
  - exp runs on ScalarE (one ACT per [128,1024] tile) for most slots; a
    tunable subset is offloaded to VectorE via range-reduced 2^f with an fp16
    round-to-int trick + quadratic Horner + int16 exponent insertion, keeping
    BOTH engines ~equally busy (ScalarE alone is the roofline otherwise).
  - Two accumulating matmuls per slot fold V'^T @ P^T into O'^T [65, 512]
    PSUM (row 64 = sum of exp). Out-matmuls trail their exp by 2 slots
    (ScalarE) / 4+ slots (VectorE, queued at unit end) so the in-order PE
    queue never waits on an activation.
  - O'^T goes PSUM -> SBUF (VectorE) -> DRAM unnormalized; the host divides
    by the denominator row and transposes during unsharding.
"""

import math

import numpy as np

import concourse.bass as bass
from concourse import bacc
import concourse.mybir as mybir
import concourse.tile as tile
from concourse.bass_utils import run_bass_kernel_spmd

B, T, H, D = 2, 2048, 16, 64
N_CORES = 8
HPC = (B * H) // N_CORES  # heads per core = 4
P = 128
NS = T // P  # 16 s-tiles
NQ = 4  # t-quarters
TW = T // NQ  # 512
SCALE = 1.0 / math.sqrt(D)
F32 = mybir.dt.float32
F16 = mybir.dt.float16
I16 = mybir.dt.int16

# Combined per-head input layout (per partition): [ K^T 1024 | Q^T 2048 | V' 1040 ]
# K first so head 0's split DMA (K + Q-quarter-0) lands fast and the first
# score matmuls start ~4us earlier.
KT_OFF = 0
QT_OFF = (NS // 2) * P
VP_OFF = QT_OFF + T
INP_W = VP_OFF + NS * (D + 1)
SPLIT0 = QT_OFF + TW  # head-0 first-chunk boundary: all K + Q quarter 0

# exp(x) = 2^y, y = x*log2(e):  z = fp16(y + 1536) holds round(y) in its low
# mantissa bits (ULP(1536)=1);  f = y - round(y) in [-0.5,0.5];
# 2^f ~ EC0 + f*(EC1 + f*EC2) (rel err 2.7e-3); result bits = (z-0x6600)<<10 + bits(p).
LOG2E = 1.0 / math.log(2.0)
EMAG = 1536.0
EMAG_BITS = 0x6600
EC0, EC1, EC2 = 1.0003519050774814, 0.7012841020239176, 0.23734859028501318

# Slots whose exp runs on VectorE: (unit, j) with unit = head*NQ + quarter.
# One per unit, early (j=1) so the ~5us DVE chain finishes before its
# out-matmuls pop at the unit boundary; every 4th unit gets a second at j=5.
DVE_JS = {u: (1,) for u in range(HPC * NQ)}

LAST_RESULT = None  # BassKernelResults of the most recent kernel() call
_BASS_CACHE = {}


def _build_bass():
    nc = bacc.Bacc("TRN2", target_bir_lowering=False)

    inp_d = nc.declare_dram_parameter("inp", [HPC, P, INP_W], F16, isOutput=False)
    out_d = nc.declare_dram_parameter("out", [HPC, NQ, D + 1, TW], F32, isOutput=True)

    with tile.TileContext(nc) as tc:
        with (
            tc.tile_pool(name="heads", bufs=3) as hpool,
            tc.tile_pool(name="pt", bufs=11) as ptpool,
            tc.tile_pool(name="outs", bufs=2) as opool,
            tc.tile_pool(name="dve", bufs=2) as dpool,
            tc.tile_pool(name="spsum", bufs=3, space="PSUM") as spsum,
            tc.tile_pool(name="opsum", bufs=2, space="PSUM") as opsum,
        ):
            # Dummy exp ACT up front: walrus inserts the ~2.7us ACT_TABLE_LOAD
            # before it, overlapping the first input DMA instead of the first
            # real activation.
            wu = opool.tile([P, 640], F16, tag="wu")
            nc.gpsimd.memset(wu[:], 0.0)
            wua = opool.tile([P, 64], F16, tag="wua")
            nc.scalar.activation(wua[:], wu[:, 0:64], mybir.ActivationFunctionType.Exp)
            # PE warm-up while the first input DMA is in flight (HAM clock-gate).
            for _w in range(3):
                wups = spsum.tile([P, 2 * TW], F32, tag="s2")
                nc.tensor.matmul(
                    wups[:, 0:TW],
                    lhsT=wu[0:64, 0:P],
                    rhs=wu[0:64, P : P + TW],
                    start=True,
                    stop=True,
                )

            inp_tiles = [
                hpool.tile([P, INP_W], F16, tag="inp", name=f"inp_sb{i}")
                for i in range(HPC)
            ]
            # Head 0's K + Q-quarter-0 in a small dedicated tile: its ~1.1us
            # DMA unblocks the first score matmuls ~3.5us before the full
            # 1 MB head-0 DMA lands (tile-granular dependency tracking).
            inp0a = hpool.tile([P, SPLIT0], F16, tag="inp0a")
            dma0a = nc.sync.dma_start(inp0a[:], inp_d.ap()[0][:, 0:SPLIT0])
            dma_issued = [False] * HPC
            dma_chain = [dma0a]

            def issue_inp_dma(hh, chain=False):
                if 0 <= hh < HPC and not dma_issued[hh]:
                    dma_issued[hh] = True
                    di = nc.sync.dma_start(inp_tiles[hh][:], inp_d.ap()[hh])
                    if chain:
                        # Serialize behind the previous input DMA: the early
                        # transfers otherwise share bandwidth and the
                        # first-chunk tile lands 3x later.
                        tile.add_dep_helper(
                            di.ins, dma_chain[-1].ins, sync=True,
                            reason="serialize input prefetch DMAs",
                        )
                        dma_chain.append(di)

            issue_inp_dma(0, chain=True)
            issue_inp_dma(1, chain=True)

            def kt_sb(hh, j):  # packed K^T chunk j: [128, 128]
                src = inp0a if hh == 0 else inp_tiles[hh]
                return src[:, KT_OFF + j * P : KT_OFF + (j + 1) * P]

            def vp_sb(hh, i):  # V' s-tile i: [128, 65]
                return inp_tiles[hh][:, VP_OFF + i * (D + 1) : VP_OFF + (i + 1) * (D + 1)]

            def emit_exp_dve(s2, pt2):
                """pt2 = exp(s2) on VectorE (fp16 2^f with int16 exponent insert).

                Reads the PSUM scores exactly ONCE (op 1) so the s2 bank frees
                fast — holding it longer stalls the scores matmul 3 slots on
                (spsum bufs=3 WAR)."""
                y = dpool.tile([P, 2 * TW], F16, tag="ey")
                nc.vector.tensor_scalar_mul(y[:], s2[:], LOG2E)
                # z = fp16(y + 1536): low mantissa bits hold round(y) (ULP=1).
                z = dpool.tile([P, 2 * TW], F16, tag="ez")
                nc.vector.tensor_scalar_add(z[:], y[:], EMAG)
                kf = dpool.tile([P, 2 * TW], F16, tag="ek")
                nc.vector.tensor_scalar_sub(kf[:], z[:], EMAG)
                f = dpool.tile([P, 2 * TW], F16, tag="ef")
                nc.vector.tensor_sub(f[:], y[:], kf[:])
                t1 = dpool.tile([P, 2 * TW], F16, tag="et")
                nc.vector.tensor_scalar(
                    t1[:], f[:], EC2, EC1,
                    mybir.AluOpType.mult, mybir.AluOpType.add,
                )
                p1 = dpool.tile([P, 2 * TW], F16, tag="ep")
                nc.vector.tensor_tensor(p1[:], f[:], t1[:], mybir.AluOpType.mult)
                pp = dpool.tile([P, 2 * TW], F16, tag="eq")
                nc.vector.tensor_scalar_add(pp[:], p1[:], EC0)
                # (z_i - 0x6600) << 10 == z_i << 10 (mod 2^16): 0x6600's low 6
                # bits are 0, so the subtrahend shifts out entirely.
                w = dpool.tile([P, 2 * TW], I16, tag="ew")
                nc.vector.tensor_scalar(
                    w[:], z.bitcast(I16)[:], 10, None,
                    mybir.AluOpType.logical_shift_left,
                )
                nc.vector.tensor_tensor(
                    pt2.bitcast(I16)[:], w[:], pp.bitcast(I16)[:], mybir.AluOpType.add
                )

            # --- slot schedule -------------------------------------------
            units = [(hh, q) for hh in range(HPC) for q in range(NQ)]
            NJ = NS // 2  # 8 slots per unit
            pending = []  # out-jobs: dicts
            emitted_count = {}  # unit -> number of out-jobs emitted

            def emit_out(job):
                u = job["u"]
                hh, q = units[u]
                first = emitted_count[u] == 0
                emitted_count[u] += 1
                last = emitted_count[u] == NJ
                j = job["j"]
                ps_o = job["ps_o"]
                pt2 = job["pt"]
                nc.tensor.matmul(
                    ps_o[:],
                    lhsT=vp_sb(hh, 2 * j),
                    rhs=pt2[:, 0:TW],
                    start=first,
                    stop=False,
                )
                nc.tensor.matmul(
                    ps_o[:],
                    lhsT=vp_sb(hh, 2 * j + 1),
                    rhs=pt2[:, TW : 2 * TW],
                    start=False,
                    stop=last,
                )
                if last:
                    o_sb = opool.tile([D + 1, TW], F32, tag="osb")
                    nc.vector.tensor_copy(o_sb[:], ps_o[:])
                    nc.sync.dma_start(out_d.ap()[hh, q], o_sb[:])

            slot_idx = 0
            for u, (hh, q) in enumerate(units):
                if q == 0:
                    issue_inp_dma(hh + 1)
                ps_o = opsum.tile([D + 1, TW], F32, tag="po")
                emitted_count[u] = 0
                qsl = slice(QT_OFF + q * TW, QT_OFF + (q + 1) * TW)
                qsrc = inp0a if (hh == 0 and q == 0) else inp_tiles[hh]
                dve_jobs = []
                for jg in range(NJ // 2):  # slot pairs: scores back-to-back
                    group = (2 * jg, 2 * jg + 1)
                    s2s = {}
                    for j in group:
                        s2 = spsum.tile([P, 2 * TW], F32, tag="s2")
                        nc.tensor.matmul(
                            s2[:, 0:TW],
                            lhsT=kt_sb(hh, j)[0:64, :],
                            rhs=qsrc[0:64, qsl],
                            start=True,
                            stop=True,
                        )
                        nc.tensor.matmul(
                            s2[:, TW : 2 * TW],
                            lhsT=kt_sb(hh, j)[64:128, :],
                            rhs=qsrc[64:128, qsl],
                            start=True,
                            stop=True,
                        )
                        s2s[j] = s2
                    for j in group:
                        pt2 = ptpool.tile([P, 2 * TW], F16, tag="pt")
                        is_dve = j in DVE_JS.get(u, ())
                        if is_dve:
                            emit_exp_dve(s2s[j], pt2)
                        else:
                            nc.scalar.activation(
                                pt2[:], s2s[j][:], mybir.ActivationFunctionType.Exp
                            )
                        job = {"u": u, "j": j, "pt": pt2, "ps_o": ps_o,
                               "after": slot_idx + (9 if is_dve else 5)}
                        if is_dve:
                            dve_jobs.append(job)
                        else:
                            pending.append(job)
                        slot_idx += 1
                    while pending and pending[0]["after"] <= slot_idx:
                        emit_out(pending.pop(0))
                pending.extend(dve_jobs)  # DVE slots drain at unit end
            while pending:
                emit_out(pending.pop(0))

    nc.compile()
    return nc


def get_bass():
    if "nc" not in _BASS_CACHE:
        _BASS_CACHE["nc"] = _build_bass()
    return _BASS_CACHE["nc"]


def make_core_inputs(q, kv, core):
    """Host-side sharding + layout for one core: returns {inp}."""
    b = core // (N_CORES // B)
    h0 = HPC * (core % (N_CORES // B))
    inp = np.empty((HPC, P, INP_W), np.float16)
    for i in range(HPC):
        h = h0 + i
        Qt = np.ascontiguousarray(q[b, :, h, :].T)  # [64, 2048]
        inp[i, :64, QT_OFF : QT_OFF + T] = Qt
        inp[i, 64:, QT_OFF : QT_OFF + T] = Qt
        Kt = (kv[b, :, 0, h, :].astype(np.float32) * SCALE).T  # [64, 2048]
        Kts = Kt.reshape(64, NS, P)
        kt = inp[i, :, KT_OFF:QT_OFF].reshape(P, NS // 2, P)
        kt[:64] = Kts[:, 0::2]  # even s-tiles -> partitions 0-63
        kt[64:] = Kts[:, 1::2]  # odd s-tiles -> partitions 64-127
        V = kv[b, :, 1, h, :].reshape(NS, P, D)  # [s_tile, p, d]
        vp = inp[i, :, VP_OFF:].reshape(P, NS, D + 1)
        vp[:, :, :D] = V.transpose(1, 0, 2)
        vp[:, :, D] = 1.0
    return {"inp": inp}


def kernel(q, kv):
    global LAST_RESULT
    q = np.asarray(q, dtype=np.float32)
    kv = np.asarray(kv, dtype=np.float32)
    assert q.shape == (B, T, H, D) and kv.shape == (B, T, 2, H, D)

    nc = get_bass()
    in_maps = [make_core_inputs(q, kv, c) for c in range(N_CORES)]
    res = run_bass_kernel_spmd(nc, in_maps, core_ids=list(range(N_CORES)))
    LAST_RESULT = res

    out = np.empty((B, T, H, D), np.float32)
    for c in range(N_CORES):
        b = c // (N_CORES // B)
        h0 = HPC * (c % (N_CORES // B))
        o = res.results[c]["out"]  # [HPC, NQ, 65, TW] unnormalized O'^T
        for i in range(HPC):
            num = o[i, :, :D, :]  # [NQ, 64, TW]
            den = o[i, :, D : D + 1, :]  # [NQ, 1, TW]
            out[b, :, h0 + i, :] = (num / den).transpose(0, 2, 1).reshape(T, D)
    return out


# revision 30
# speedup vs baseline: 1.2238x; 1.2238x over previous
"""Cross-attention kernel for Trainium2 (8 NeuronCores, SPMD).

Problem: q [2, 2048, 16, 64], kv [2, 2048, 2, 16, 64] (k=kv[:,:,0], v=kv[:,:,1])
  scores = einsum('bthd,bshd->bhts', q, k/sqrt(d)); P = softmax(scores, -1)
  out = einsum('bhts,bshd->bthd', P, v)    -> [2, 2048, 16, 64]

Sharding: 32 (b,h) heads across 8 cores -> 4 heads/core (data parallel on b,
tensor parallel on h; no communication).

Per-core algorithm (per head, t=s=2048, d=64), v3 pipeline:
  - Host lays out one fp16 tensor per head: Q^T [64,2048] duplicated into both
    PE row halves, K^T*scale packed (even s-tiles partitions 0-63, odd 64-127;
    2-way row-packed score matmuls), V' = [V, 1] per s-tile (ones column gives
    the softmax denominator for free in the same matmul).
  - 128 slots = (head, t-quarter, s-tile-pair). Per slot: two row-packed
    matmuls -> S^T halves of one [128,1024] PSUM tile; exp -> P^T fp16.
  - exp runs on ScalarE (one ACT per [128,1024] tile) for most slots; a
    tunable subset is offloaded to VectorE via range-reduced 2^f with an fp16
    round-to-int trick + quadratic Horner + int16 exponent insertion, keeping
    BOTH engines ~equally busy (ScalarE alone is the roofline otherwise).
  - Two accumulating matmuls per slot fold V'^T @ P^T into O'^T [65, 512]
    PSUM (row 64 = sum of exp). Out-matmuls trail their exp by 2 slots
    (ScalarE) / 4+ slots (VectorE, queued at unit end) so the in-order PE
    queue never waits on an activation.
  - O'^T goes PSUM -> SBUF (VectorE) -> DRAM unnormalized; the host divides
    by the denominator row and transposes during unsharding.
"""

import math

import numpy as np

import concourse.bass as bass
from concourse import bacc
import concourse.mybir as mybir
import concourse.tile as tile
from concourse.bass_utils import run_bass_kernel_spmd

B, T, H, D = 2, 2048, 16, 64
N_CORES = 8
HPC = (B * H) // N_CORES  # heads per core = 4
P = 128
NS = T // P  # 16 s-tiles
NQ = 4  # t-quarters
TW = T // NQ  # 512
SCALE = 1.0 / math.sqrt(D)
F32 = mybir.dt.float32
F16 = mybir.dt.float16
I16 = mybir.dt.int16

# Combined per-head input layout (per partition): [ K^T 1024 | Q^T 2048 | V' 1040 ]
# K first so head 0's split DMA (K + Q-quarter-0) lands fast and the first
# score matmuls start ~4us earlier.
KT_OFF = 0
QT_OFF = (NS // 2) * P
VP_OFF = QT_OFF + T
INP_W = VP_OFF + NS * (D + 1)
SPLIT0 = QT_OFF + TW  # head-0 first-chunk boundary: all K + Q quarter 0

# exp(x) = 2^y, y = x*log2(e):  z = fp16(y + 1536) holds round(y) in its low
# mantissa bits (ULP(1536)=1);  f = y - round(y) in [-0.5,0.5];
# 2^f ~ EC0 + f*(EC1 + f*EC2) (rel err 2.7e-3); result bits = (z-0x6600)<<10 + bits(p).
LOG2E = 1.0 / math.log(2.0)
EMAG = 1536.0
EMAG_BITS = 0x6600
EC0, EC1, EC2 = 1.0003519050774814, 0.7012841020239176, 0.23734859028501318

# Slots whose exp runs on VectorE: (unit, j) with unit = head*NQ + quarter.
# One per unit, early (j=1) so the ~5us DVE chain finishes before its
# out-matmuls pop at the unit boundary; every 4th unit gets a second at j=5.
DVE_JS = {u: (1,) for u in range(HPC * NQ)}

LAST_RESULT = None  # BassKernelResults of the most recent kernel() call
_BASS_CACHE = {}


def _build_bass():
    nc = bacc.Bacc("TRN2", target_bir_lowering=False)

    inp_d = nc.declare_dram_parameter("inp", [HPC, P, INP_W], F16, isOutput=False)
    out_d = nc.declare_dram_parameter("out", [HPC, NQ, D + 1, TW], F32, isOutput=True)

    with tile.TileContext(nc) as tc:
        with (
            tc.tile_pool(name="heads", bufs=3) as hpool,
            tc.tile_pool(name="pt", bufs=11) as ptpool,
            tc.tile_pool(name="outs", bufs=2) as opool,
            tc.tile_pool(name="dve", bufs=2) as dpool,
            tc.tile_pool(name="spsum", bufs=3, space="PSUM") as spsum,
            tc.tile_pool(name="opsum", bufs=2, space="PSUM") as opsum,
        ):
            # Dummy exp ACT up front: walrus inserts the ~2.7us ACT_TABLE_LOAD
            # before it, overlapping the first input DMA instead of the first
            # real activation.
            wu = opool.tile([P, 640], F16, tag="wu")
            nc.gpsimd.memset(wu[:], 0.0)
            wua = opool.tile([P, 64], F16, tag="wua")
            nc.scalar.activation(wua[:], wu[:, 0:64], mybir.ActivationFunctionType.Exp)
            # PE warm-up while the first input DMA is in flight (HAM clock-gate).
            for _w in range(6):
                wups = spsum.tile([P, 2 * TW], F32, tag="s2")
                nc.tensor.matmul(
                    wups[:, 0:TW],
                    lhsT=wu[0:64, 0:P],
                    rhs=wu[0:64, P : P + TW],
                    start=True,
                    stop=True,
                )

            inp_tiles = [
                hpool.tile([P, INP_W], F16, tag="inp", name=f"inp_sb{i}")
                for i in range(HPC)
            ]
            dma_issued = [False] * HPC

            def issue_inp_dma(hh):
                if 0 <= hh < HPC and not dma_issued[hh]:
                    dma_issued[hh] = True
                    if hh == 0:
                        nc.sync.dma_start(
                            inp_tiles[0][:, 0:SPLIT0], inp_d.ap()[0][:, 0:SPLIT0]
                        )
                        nc.sync.dma_start(
                            inp_tiles[0][:, SPLIT0:INP_W],
                            inp_d.ap()[0][:, SPLIT0:INP_W],
                        )
                    else:
                        nc.sync.dma_start(inp_tiles[hh][:], inp_d.ap()[hh])

            issue_inp_dma(0)
            issue_inp_dma(1)

            def kt_sb(hh, j):  # packed K^T chunk j: [128, 128]
                return inp_tiles[hh][:, KT_OFF + j * P : KT_OFF + (j + 1) * P]

            def vp_sb(hh, i):  # V' s-tile i: [128, 65]
                return inp_tiles[hh][:, VP_OFF + i * (D + 1) : VP_OFF + (i + 1) * (D + 1)]

            def emit_exp_dve(s2, pt2):
                """pt2 = exp(s2) on VectorE (fp16 2^f with int16 exponent insert).

                Reads the PSUM scores exactly ONCE (op 1) so the s2 bank frees
                fast — holding it longer stalls the scores matmul 3 slots on
                (spsum bufs=3 WAR)."""
                y = dpool.tile([P, 2 * TW], F16, tag="ey")
                nc.vector.tensor_scalar_mul(y[:], s2[:], LOG2E)
                # z = fp16(y + 1536): low mantissa bits hold round(y) (ULP=1).
                z = dpool.tile([P, 2 * TW], F16, tag="ez")
                nc.vector.tensor_scalar_add(z[:], y[:], EMAG)
                kf = dpool.tile([P, 2 * TW], F16, tag="ek")
                nc.vector.tensor_scalar_sub(kf[:], z[:], EMAG)
                f = dpool.tile([P, 2 * TW], F16, tag="ef")
                nc.vector.tensor_sub(f[:], y[:], kf[:])
                t1 = dpool.tile([P, 2 * TW], F16, tag="et")
                nc.vector.tensor_scalar(
                    t1[:], f[:], EC2, EC1,
                    mybir.AluOpType.mult, mybir.AluOpType.add,
                )
                p1 = dpool.tile([P, 2 * TW], F16, tag="ep")
                nc.vector.tensor_tensor(p1[:], f[:], t1[:], mybir.AluOpType.mult)
                pp = dpool.tile([P, 2 * TW], F16, tag="eq")
                nc.vector.tensor_scalar_add(pp[:], p1[:], EC0)
                # (z_i - 0x6600) << 10 == z_i << 10 (mod 2^16): 0x6600's low 6
                # bits are 0, so the subtrahend shifts out entirely.
                w = dpool.tile([P, 2 * TW], I16, tag="ew")
                nc.vector.tensor_scalar(
                    w[:], z.bitcast(I16)[:], 10, None,
                    mybir.AluOpType.logical_shift_left,
                )
                nc.vector.tensor_tensor(
                    pt2.bitcast(I16)[:], w[:], pp.bitcast(I16)[:], mybir.AluOpType.add
                )

            # --- slot schedule -------------------------------------------
            units = [(hh, q) for hh in range(HPC) for q in range(NQ)]
            NJ = NS // 2  # 8 slots per unit
            pending = []  # out-jobs: dicts
            emitted_count = {}  # unit -> number of out-jobs emitted

            def emit_out(job):
                u = job["u"]
                hh, q = units[u]
                first = emitted_count[u] == 0
                emitted_count[u] += 1
                last = emitted_count[u] == NJ
                j = job["j"]
                ps_o = job["ps_o"]
                pt2 = job["pt"]
                nc.tensor.matmul(
                    ps_o[:],
                    lhsT=vp_sb(hh, 2 * j),
                    rhs=pt2[:, 0:TW],
                    start=first,
                    stop=False,
                )
                nc.tensor.matmul(
                    ps_o[:],
                    lhsT=vp_sb(hh, 2 * j + 1),
                    rhs=pt2[:, TW : 2 * TW],
                    start=False,
                    stop=last,
                )
                if last:
                    o_sb = opool.tile([D + 1, TW], F32, tag="osb")
                    nc.vector.tensor_copy(o_sb[:], ps_o[:])
                    nc.sync.dma_start(out_d.ap()[hh, q], o_sb[:])

            slot_idx = 0
            for u, (hh, q) in enumerate(units):
                if q == 0:
                    issue_inp_dma(hh + 1)
                ps_o = opsum.tile([D + 1, TW], F32, tag="po")
                emitted_count[u] = 0
                qsl = slice(QT_OFF + q * TW, QT_OFF + (q + 1) * TW)
                qsrc = inp_tiles[hh]
                dve_jobs = []
                for jg in range(NJ // 2):  # slot pairs: scores back-to-back
                    group = (2 * jg, 2 * jg + 1)
                    s2s = {}
                    for j in group:
                        s2 = spsum.tile([P, 2 * TW], F32, tag="s2")
                        nc.tensor.matmul(
                            s2[:, 0:TW],
                            lhsT=kt_sb(hh, j)[0:64, :],
                            rhs=qsrc[0:64, qsl],
                            start=True,
                            stop=True,
                        )
                        nc.tensor.matmul(
                            s2[:, TW : 2 * TW],
                            lhsT=kt_sb(hh, j)[64:128, :],
                            rhs=qsrc[64:128, qsl],
                            start=True,
                            stop=True,
                        )
                        s2s[j] = s2
                    for j in group:
                        pt2 = ptpool.tile([P, 2 * TW], F16, tag="pt")
                        is_dve = j in DVE_JS.get(u, ())
                        if is_dve:
                            emit_exp_dve(s2s[j], pt2)
                        else:
                            nc.scalar.activation(
                                pt2[:], s2s[j][:], mybir.ActivationFunctionType.Exp
                            )
                        job = {"u": u, "j": j, "pt": pt2, "ps_o": ps_o,
                               "after": slot_idx + (9 if is_dve else 5)}
                        if is_dve:
                            dve_jobs.append(job)
                        else:
                            pending.append(job)
                        slot_idx += 1
                    while pending and pending[0]["after"] <= slot_idx:
                        emit_out(pending.pop(0))
                pending.extend(dve_jobs)  # DVE slots drain at unit end
            while pending:
                emit_out(pending.pop(0))

    nc.compile()
    return nc


def get_bass():
    if "nc" not in _BASS_CACHE:
        _BASS_CACHE["nc"] = _build_bass()
    return _BASS_CACHE["nc"]


def make_core_inputs(q, kv, core):
    """Host-side sharding + layout for one core: returns {inp}."""
    b = core // (N_CORES // B)
    h0 = HPC * (core % (N_CORES // B))
    inp = np.empty((HPC, P, INP_W), np.float16)
    for i in range(HPC):
        h = h0 + i
        Qt = np.ascontiguousarray(q[b, :, h, :].T)  # [64, 2048]
        inp[i, :64, QT_OFF : QT_OFF + T] = Qt
        inp[i, 64:, QT_OFF : QT_OFF + T] = Qt
        Kt = (kv[b, :, 0, h, :].astype(np.float32) * SCALE).T  # [64, 2048]
        Kts = Kt.reshape(64, NS, P)
        kt = inp[i, :, KT_OFF:QT_OFF].reshape(P, NS // 2, P)
        kt[:64] = Kts[:, 0::2]  # even s-tiles -> partitions 0-63
        kt[64:] = Kts[:, 1::2]  # odd s-tiles -> partitions 64-127
        V = kv[b, :, 1, h, :].reshape(NS, P, D)  # [s_tile, p, d]
        vp = inp[i, :, VP_OFF:].reshape(P, NS, D + 1)
        vp[:, :, :D] = V.transpose(1, 0, 2)
        vp[:, :, D] = 1.0
    return {"inp": inp}


def kernel(q, kv):
    global LAST_RESULT
    q = np.asarray(q, dtype=np.float32)
    kv = np.asarray(kv, dtype=np.float32)
    assert q.shape == (B, T, H, D) and kv.shape == (B, T, 2, H, D)

    nc = get_bass()
    in_maps = [make_core_inputs(q, kv, c) for c in range(N_CORES)]
    res = run_bass_kernel_spmd(nc, in_maps, core_ids=list(range(N_CORES)))
    LAST_RESULT = res

    out = np.empty((B, T, H, D), np.float32)
    for c in range(N_CORES):
        b = c // (N_CORES // B)
        h0 = HPC * (c % (N_CORES // B))
        o = res.results[c]["out"]  # [HPC, NQ, 65, TW] unnormalized O'^T
        for i in range(HPC):
            num = o[i, :, :D, :]  # [NQ, 64, TW]
            den = o[i, :, D : D + 1, :]  # [NQ, 1, TW]
            out[b, :, h0 + i, :] = (num / den).transpose(0, 2, 1).reshape(T, D)
    return out
